# revision 15
# baseline (speedup 1.0000x reference)
"""Trainium2 Bass kernel for BSplineEncoder.

Reference computation (per scalar x in [0,1)):
    basis = cubic B-spline basis (10 functions, knots uniform on [0,1], clamped)
    out[e, :] = basis(x_e) @ W.T + b          # W: [128, 10], b: [128]

Reformulation: on each of the 7 uniform knot intervals the spline is a cubic,
so with j = floor(7x), u = 7x - j:
    out[e, c] = sum_m u^m * C[j, m, c]
C is precomputed host-side from W (bias folded into the m=0 coefficient of
every interval — valid because exactly one interval indicator fires).
Writing v[e, 4j+m] = ind_j(e) * u^m (32-wide, zero-padded), out = v @ C.

Precision: the matmul runs as a split-fp16 product with fp32 PSUM accumulation
    out = vh@Ch + vl@Ch + vh@Cl       (vh=fp16(v), vl=fp16(v-vh); same for C)
which is fp32-accurate to ~2e-6 while streaming at 1 cycle/row (fp32 matmul
on TRN2 is a 2-instruction replay at 4 cycles/row — 8x slower).

Device pipeline (per core; elements sharded 8 ways on batch):
    x (host-preshuffled) -> [128, F] tiles -> DVE computes vh/vl fp16 feature
    tiles -> DVE 32x32 stream-transpose puts K on partitions -> PE row-tiled
    matmuls (4 groups x 3 accumulating MMs) into a 4-bank PSUM quad ->
    one strided ACT copy per slab -> contiguous 4 MiB DMA per block.

The element->(partition, lane) mapping is chosen so the stream-transpose's
block-local semantics line up with the matmul row groups AND the output DMA
stays fully contiguous; the host reorders x (cheap: 4 MiB) to compensate.
"""

import numpy as np

_P = 128            # SBUF partitions
_F = 64             # x values per partition per block (16 slabs x 4)
_NB = 16            # blocks per core
_NPC = _P * _F * _NB  # 131072 elements per core
_NCORES = 8
_CH = 128           # output channels
_NS = 16            # slabs per block (each slab: 512 elements)
_DEGREE = 3
_N_BASIS = 10
# largest fp32 strictly below 7.0: keeps j <= 6 and u < 1 for any x
_CLAMP_HI = float(np.nextafter(np.float32(7.0), np.float32(0.0)))


# ---------------------------------------------------------------- host math
def _bspline_basis_1d_f64(x, knots, degree=_DEGREE):
    n_knots = knots.shape[0]
    B = ((x[None, :] >= knots[:-1, None]) & (x[None, :] < knots[1:, None])).astype(
        np.float64
    )
    for d in range(1, degree + 1):
        n = n_knots - d - 1
        k_i = knots[:n]
        k_i1 = knots[1 : n + 1]
        k_id = knots[d : d + n]
        k_id1 = knots[d + 1 : d + 1 + n]
        denom1 = k_id - k_i
        denom2 = k_id1 - k_i1
        safe1 = np.where(denom1 != 0, denom1, 1.0)[:, None]
        safe2 = np.where(denom2 != 0, denom2, 1.0)[:, None]
        t1 = np.where(
            denom1[:, None] != 0, (x[None, :] - k_i[:, None]) / safe1 * B[:n], 0.0
        )
        t2 = np.where(
            denom2[:, None] != 0,
            (k_id1[:, None] - x[None, :]) / safe2 * B[1 : n + 1],
            0.0,
        )
        B = t1 + t2
    return B.T  # [N, n_basis]


def _interval_poly_coeffs():
    """M[j][r, m]: basis_{j+r}(x) = sum_m M[j][r,m] * u^m on interval j."""
    n_knots = _N_BASIS + _DEGREE + 1
    base = np.linspace(0.0, 1.0, n_knots - 2 * _DEGREE)
    knots = np.concatenate(
        [np.repeat(base[:1], _DEGREE), base, np.repeat(base[-1:], _DEGREE)]
    )
    M = np.zeros((7, 4, 4))
    us = np.array([0.1, 0.35, 0.65, 0.9])
    V = np.vander(us, 4, increasing=True)
    for j in range(7):
        xs = (j + us) / 7.0
        Bv = _bspline_basis_1d_f64(xs, knots)  # [4, 10]
        coef = np.linalg.solve(V, Bv[:, j : j + 4])  # [m, r]
        M[j] = coef.T  # [r, m]
    return M


def _build_c32(W, b):
    """[32, 128] f32 coeff matrix: rows 4j+m hold sum_r M[j][r,m] W[:, j+r]
    (+ b folded into m=0); rows 28..31 zero."""
    M = _interval_poly_coeffs()
    W64 = np.asarray(W, dtype=np.float64)
    b64 = np.asarray(b, dtype=np.float64)
    C = np.zeros((32, W64.shape[0]))
    for j in range(7):
        for m in range(4):
            C[4 * j + m] = M[j][:, m] @ W64[:, j : j + 4].T
        C[4 * j + 0] += b64
    return C.astype(np.float32)


def _build_cmats(W, b):
    """(ch_rep, cl_rep): fp16 high/low split of C, 4x vertically replicated."""
    C = _build_c32(W, b)
    Ch = C.astype(np.float16)
    Cl = (C - Ch.astype(np.float32)).astype(np.float16)
    return (
        np.ascontiguousarray(np.tile(Ch, (4, 1))),
        np.ascontiguousarray(np.tile(Cl, (4, 1))),
    )


def _shuffle_x(x_core, nb=_NB):
    """Reorder a core's flat x so that compute partition q=32*fl+i, free col
    4*s+a holds element e = b*8192 + s*512 + 4*(32a+i) + fl."""
    # x[b, s, a, i, fl] -> xh[b, (fl,i), (s,a)]
    xs = x_core.reshape(nb, _NS, 4, 32, 4)
    return np.ascontiguousarray(xs.transpose(0, 4, 3, 1, 2).reshape(nb, 128, _F))


# ---------------------------------------------------------------- bass build
def build_nc(nb=_NB):
    """Build the (single-core SPMD) Bass module. Same NEFF runs on all cores."""
    from concourse import bacc, mybir, tile

    Alu = mybir.AluOpType
    f32 = mybir.dt.float32
    f16 = mybir.dt.float16
    npc = _P * _F * nb
    f = _F

    nc = bacc.Bacc("TRN2", target_bir_lowering=False, debug=False)

    x_d = nc.dram_tensor("x", [nb, 128, f], f32, kind="ExternalInput")
    ch_d = nc.dram_tensor("cmath", [128, 128], f16, kind="ExternalInput")
    cl_d = nc.dram_tensor("cmatl", [128, 128], f16, kind="ExternalInput")
    out_d = nc.dram_tensor("out", [npc, _CH], f32, kind="ExternalOutput")
    # interval thresholds 1..8 (g_c = t7 >= c); g_8 is always 0 (t7 < 7)
    iota_d = nc.inline_tensor(
        np.tile(np.arange(1, 9, dtype=np.float32), (128, 1)).copy(), name="iota8"
    )

    # DRAM row for element = b*8192 + s*512 + 4p + fl; sbuf outb free col is
    # s*512 + fl*128 + ch -> per-block AP [128, 8192] with 2 KiB runs
    out_r = out_d.ap().rearrange(
        "(b s p fl) c -> b p s fl c", b=nb, s=_NS, p=_P, fl=4
    )

    with tile.TileContext(nc) as tc:
        with (
            tc.tile_pool(name="consts", bufs=1) as cpool,
            tc.tile_pool(name="xp", bufs=3) as xpool,
            tc.tile_pool(name="sm", bufs=2) as smpool,
            tc.tile_pool(name="vv", bufs=2) as vpool,
            tc.tile_pool(name="vt", bufs=6) as vtpool,
            tc.tile_pool(name="ob", bufs=2) as opool,
            tc.tile_pool(name="pso", bufs=2, space="PSUM") as psum_o,
        ):
            ch_t = cpool.tile([128, 128], f16)
            cl_t = cpool.tile([128, 128], f16)
            iota8 = cpool.tile([128, 8], f32)
            nc.sync.dma_start(ch_t[:], ch_d.ap())
            nc.sync.dma_start(cl_t[:], cl_d.ap())
            nc.sync.dma_start(iota8[:], iota_d.ap())

            for bi in range(nb):
                x_t = xpool.tile([_P, f], f32, tag="x")
                nc.sync.dma_start(x_t[:], x_d.ap()[bi])

                # t7 = min(7*x, 6.9999995)
                t7 = smpool.tile([_P, f], f32, tag="t7")
                nc.vector.tensor_scalar(
                    t7[:], x_t[:], 7.0, _CLAMP_HI, Alu.mult, Alu.min
                )
                # g[p, f, c]: c=0 -> 1.0; c=1..8 -> (t7 >= c)  (fp16: exact 0/1)
                g_t = smpool.tile([_P, f, 9], f16, tag="g")
                nc.gpsimd.memset(g_t[:, :, 0], 1.0)
                nc.vector.tensor_tensor(
                    g_t[:, :, 1:9],
                    t7[:].unsqueeze(2).broadcast_to([_P, f, 8]),
                    iota8[:].unsqueeze(1).broadcast_to([_P, f, 8]),
                    Alu.is_ge,
                )
                j_t = smpool.tile([_P, f], f32, tag="j")
                nc.vector.tensor_reduce(
                    j_t[:], g_t[:, :, 1:9], mybir.AxisListType.X, Alu.add
                )
                # pow[p, f, m] = u^m in fp32;  u = t7 - j
                pw = smpool.tile([_P, f, 4], f32, tag="pow")
                nc.vector.tensor_tensor(pw[:, :, 1], t7[:], j_t[:], Alu.subtract)
                nc.gpsimd.tensor_tensor(pw[:, :, 2], pw[:, :, 1], pw[:, :, 1], Alu.mult)
                nc.gpsimd.tensor_tensor(pw[:, :, 3], pw[:, :, 2], pw[:, :, 1], Alu.mult)
                nc.gpsimd.memset(pw[:, :, 0], 1.0)
                # fp16 high/low split of pow
                pwh = smpool.tile([_P, f, 4], f16, tag="pwh")
                nc.gpsimd.tensor_copy(pwh[:], pw[:])
                pwl = smpool.tile([_P, f, 4], f16, tag="pwl")
                nc.gpsimd.tensor_tensor(pwl[:], pw[:], pwh[:], Alu.subtract)
                # ind_c = g[c] - g[c+1]  (fp16; exact 0/1)
                ind = smpool.tile([_P, f, 8], f16, tag="ind")
                nc.gpsimd.tensor_tensor(
                    ind[:], g_t[:, :, 0:8], g_t[:, :, 1:9], Alu.subtract
                )
                # vh/vl[p, 32c + 4j' + m] = ind * pow_{h,l}
                vh = vpool.tile([_P, f, 8, 4], f16, tag="vh")
                vl = vpool.tile([_P, f, 8, 4], f16, tag="vl")
                ind_b = ind[:].unsqueeze(3).broadcast_to([_P, f, 8, 4])
                nc.vector.tensor_tensor(
                    vh[:], ind_b, pwh[:].unsqueeze(2).broadcast_to([_P, f, 8, 4]),
                    Alu.mult,
                )
                nc.gpsimd.tensor_tensor(
                    vl[:], ind_b, pwl[:].unsqueeze(2).broadcast_to([_P, f, 8, 4]),
                    Alu.mult,
                )
                vh2 = vh[:].rearrange("p f j m -> p (f j m)")
                vl2 = vl[:].rearrange("p f j m -> p (f j m)")

                outb = opool.tile([_P, f * _CH], f32, tag="outb")
                for s4 in range(_NS // 4):
                    # 32x32 block transpose puts K on partitions: row-group
                    # fl holds the 32-wide feature vecs of its 128 elements.
                    # 4 slabs per transpose op to amortize per-op overhead.
                    vht = vtpool.tile([128, 512], f16, tag="vht")
                    vlt = vtpool.tile([128, 512], f16, tag="vlt")
                    nc.vector.transpose(vht[:], vh2[:, 512 * s4 : 512 * (s4 + 1)])
                    nc.vector.transpose(vlt[:], vl2[:, 512 * s4 : 512 * (s4 + 1)])
                    for sl in range(4):
                        s = 4 * s4 + sl
                        c0, c1 = 128 * sl, 128 * sl + 128
                        # 4-bank PSUM quad; each row-group matmuls into its bank
                        quad = psum_o.tile([128, 2048], f32, tag="quad")
                        for fl in range(4):
                            o_ap = quad[:, 512 * fl : 512 * fl + 128]
                            tp = (32 * fl, 0)
                            r0, r1 = 32 * fl, 32 * fl + 32
                            # vh first+second: identical stationary operand
                            # back-to-back lets the weight path cache/elide
                            nc.tensor.matmul(
                                o_ap, vht[r0:r1, c0:c1], ch_t[r0:r1, :],
                                start=True, stop=False, tile_position=tp,
                            )
                            nc.tensor.matmul(
                                o_ap, vht[r0:r1, c0:c1], cl_t[r0:r1, :],
                                start=False, stop=False, tile_position=tp,
                            )
                            nc.tensor.matmul(
                                o_ap, vlt[r0:r1, c0:c1], ch_t[r0:r1, :],
                                start=False, stop=True, tile_position=tp,
                            )
                        # one strided copy: 4 banks -> contiguous 512 cols
                        qv = quad[:].rearrange("p (fl w) -> p fl w", fl=4)[:, :, 0:128]
                        ov = outb[:, 512 * s : 512 * (s + 1)].rearrange(
                            "p (fl c) -> p fl c", fl=4
                        )
                        nc.scalar.copy(ov, qv)
                    # half-block out DMA: start shipping as soon as the first
                    # 8 slabs are copied, overlapping the rest of the block
                    if s4 == _NS // 8 - 1 or s4 == _NS // 4 - 1:
                        h = 0 if s4 == _NS // 8 - 1 else 1
                        hs = _NS // 2
                        nc.sync.dma_start(
                            out_r[bi, :, h * hs : (h + 1) * hs],
                            outb[:, h * hs * 512 : (h + 1) * hs * 512].rearrange(
                                "p (s fl c) -> p s fl c", s=hs, fl=4
                            ),
                        )

    nc.compile()
    return nc


_NC_CACHE = None


def _get_nc():
    global _NC_CACHE
    if _NC_CACHE is None:
        _NC_CACHE = build_nc()
    return _NC_CACHE


# ---------------------------------------------------------------- entrypoint
def kernel(x, W, b):
    from concourse.bass_utils import run_bass_kernel_spmd

    x = np.asarray(x, dtype=np.float32)
    Bsz, T = x.shape
    ch_rep, cl_rep = _build_cmats(W, b)
    nc = _get_nc()

    shards = x.reshape(_NCORES, _NPC)
    in_maps = [
        {"x": _shuffle_x(shards[i]), "cmath": ch_rep, "cmatl": cl_rep}
        for i in range(_NCORES)
    ]
    res = run_bass_kernel_spmd(nc, in_maps, core_ids=list(range(_NCORES)))
    out = np.concatenate([res.results[i]["out"] for i in range(_NCORES)], axis=0)
    return out.reshape(Bsz, T, _CH)


# revision 16
# speedup vs baseline: 1.0199x; 1.0199x over previous
"""Trainium2 Bass kernel for BSplineEncoder.

Reference computation (per scalar x in [0,1)):
    basis = cubic B-spline basis (10 functions, knots uniform on [0,1], clamped)
    out[e, :] = basis(x_e) @ W.T + b          # W: [128, 10], b: [128]

Reformulation: on each of the 7 uniform knot intervals the spline is a cubic,
so with j = floor(7x), u = 7x - j:
    out[e, c] = sum_m u^m * C[j, m, c]
C is precomputed host-side from W (bias folded into the m=0 coefficient of
every interval — valid because exactly one interval indicator fires).
Writing v[e, 4j+m] = ind_j(e) * u^m (32-wide, zero-padded), out = v @ C.

Precision: the matmul runs as a split-fp16 product with fp32 PSUM accumulation
    out = vh@Ch + vl@Ch + vh@Cl       (vh=fp16(v), vl=fp16(v-vh); same for C)
which is fp32-accurate to ~2e-6 while streaming at 1 cycle/row (fp32 matmul
on TRN2 is a 2-instruction replay at 4 cycles/row — 8x slower).

Device pipeline (per core; elements sharded 8 ways on batch):
    x (host-preshuffled) -> [128, F] tiles -> DVE computes vh/vl fp16 feature
    tiles -> DVE 32x32 stream-transpose puts K on partitions -> PE row-tiled
    matmuls (4 groups x 3 accumulating MMs) into a 4-bank PSUM quad ->
    one strided ACT copy per slab -> contiguous 4 MiB DMA per block.

The element->(partition, lane) mapping is chosen so the stream-transpose's
block-local semantics line up with the matmul row groups AND the output DMA
stays fully contiguous; the host reorders x (cheap: 4 MiB) to compensate.
"""

import numpy as np

_P = 128            # SBUF partitions
_F = 64             # x values per partition per block (16 slabs x 4)
_NB = 16            # blocks per core
_NPC = _P * _F * _NB  # 131072 elements per core
_NCORES = 8
_CH = 128           # output channels
_NS = 16            # slabs per block (each slab: 512 elements)
_DEGREE = 3
_N_BASIS = 10
# largest fp32 strictly below 7.0: keeps j <= 6 and u < 1 for any x
_CLAMP_HI = float(np.nextafter(np.float32(7.0), np.float32(0.0)))


# ---------------------------------------------------------------- host math
def _bspline_basis_1d_f64(x, knots, degree=_DEGREE):
    n_knots = knots.shape[0]
    B = ((x[None, :] >= knots[:-1, None]) & (x[None, :] < knots[1:, None])).astype(
        np.float64
    )
    for d in range(1, degree + 1):
        n = n_knots - d - 1
        k_i = knots[:n]
        k_i1 = knots[1 : n + 1]
        k_id = knots[d : d + n]
        k_id1 = knots[d + 1 : d + 1 + n]
        denom1 = k_id - k_i
        denom2 = k_id1 - k_i1
        safe1 = np.where(denom1 != 0, denom1, 1.0)[:, None]
        safe2 = np.where(denom2 != 0, denom2, 1.0)[:, None]
        t1 = np.where(
            denom1[:, None] != 0, (x[None, :] - k_i[:, None]) / safe1 * B[:n], 0.0
        )
        t2 = np.where(
            denom2[:, None] != 0,
            (k_id1[:, None] - x[None, :]) / safe2 * B[1 : n + 1],
            0.0,
        )
        B = t1 + t2
    return B.T  # [N, n_basis]


def _interval_poly_coeffs():
    """M[j][r, m]: basis_{j+r}(x) = sum_m M[j][r,m] * u^m on interval j."""
    n_knots = _N_BASIS + _DEGREE + 1
    base = np.linspace(0.0, 1.0, n_knots - 2 * _DEGREE)
    knots = np.concatenate(
        [np.repeat(base[:1], _DEGREE), base, np.repeat(base[-1:], _DEGREE)]
    )
    M = np.zeros((7, 4, 4))
    us = np.array([0.1, 0.35, 0.65, 0.9])
    V = np.vander(us, 4, increasing=True)
    for j in range(7):
        xs = (j + us) / 7.0
        Bv = _bspline_basis_1d_f64(xs, knots)  # [4, 10]
        coef = np.linalg.solve(V, Bv[:, j : j + 4])  # [m, r]
        M[j] = coef.T  # [r, m]
    return M


def _build_c32(W, b):
    """[32, 128] f32 coeff matrix: rows 4j+m hold sum_r M[j][r,m] W[:, j+r]
    (+ b folded into m=0); rows 28..31 zero."""
    M = _interval_poly_coeffs()
    W64 = np.asarray(W, dtype=np.float64)
    b64 = np.asarray(b, dtype=np.float64)
    C = np.zeros((32, W64.shape[0]))
    for j in range(7):
        for m in range(4):
            C[4 * j + m] = M[j][:, m] @ W64[:, j : j + 4].T
        C[4 * j + 0] += b64
    return C.astype(np.float32)


def _build_cmats(W, b):
    """(ch_rep, cl_rep): fp16 high/low split of C, 4x vertically replicated."""
    C = _build_c32(W, b)
    Ch = C.astype(np.float16)
    Cl = (C - Ch.astype(np.float32)).astype(np.float16)
    return (
        np.ascontiguousarray(np.tile(Ch, (4, 1))),
        np.ascontiguousarray(np.tile(Cl, (4, 1))),
    )


def _shuffle_x(x_core, nb=_NB):
    """Reorder a core's flat x so that compute partition q=32*fl+i, free col
    4*s+a holds element e = b*8192 + s*512 + 4*(32a+i) + fl."""
    # x[b, s, a, i, fl] -> xh[b, (fl,i), (s,a)]
    xs = x_core.reshape(nb, _NS, 4, 32, 4)
    return np.ascontiguousarray(xs.transpose(0, 4, 3, 1, 2).reshape(nb, 128, _F))


# ---------------------------------------------------------------- bass build
def build_nc(nb=_NB):
    """Build the (single-core SPMD) Bass module. Same NEFF runs on all cores."""
    from concourse import bacc, mybir, tile

    Alu = mybir.AluOpType
    f32 = mybir.dt.float32
    f16 = mybir.dt.float16
    npc = _P * _F * nb
    f = _F

    nc = bacc.Bacc("TRN2", target_bir_lowering=False, debug=False)

    x_d = nc.dram_tensor("x", [nb, 128, f], f32, kind="ExternalInput")
    ch_d = nc.dram_tensor("cmath", [128, 128], f16, kind="ExternalInput")
    cl_d = nc.dram_tensor("cmatl", [128, 128], f16, kind="ExternalInput")
    out_d = nc.dram_tensor("out", [npc, _CH], f32, kind="ExternalOutput")
    # interval thresholds 1..8 (g_c = t7 >= c); g_8 is always 0 (t7 < 7)
    iota_d = nc.inline_tensor(
        np.tile(np.arange(1, 9, dtype=np.float32), (128, 1)).copy(), name="iota8"
    )

    # DRAM row for element = b*8192 + s*512 + 4p + fl; sbuf outb free col is
    # s*512 + fl*128 + ch -> per-block AP [128, 8192] with 2 KiB runs
    out_r = out_d.ap().rearrange(
        "(b s p fl) c -> b p s fl c", b=nb, s=_NS, p=_P, fl=4
    )

    with tile.TileContext(nc) as tc:
        with (
            tc.tile_pool(name="consts", bufs=1) as cpool,
            tc.tile_pool(name="xp", bufs=3) as xpool,
            tc.tile_pool(name="sm", bufs=2) as smpool,
            tc.tile_pool(name="vv", bufs=2) as vpool,
            tc.tile_pool(name="vt", bufs=6) as vtpool,
            tc.tile_pool(name="ob", bufs=2) as opool,
            tc.tile_pool(name="pso", bufs=2, space="PSUM") as psum_o,
        ):
            ch_t = cpool.tile([128, 128], f16)
            cl_t = cpool.tile([128, 128], f16)
            iota8 = cpool.tile([128, 8], f32)
            nc.sync.dma_start(ch_t[:], ch_d.ap())
            nc.sync.dma_start(cl_t[:], cl_d.ap())
            nc.sync.dma_start(iota8[:], iota_d.ap())

            for bi in range(nb):
                x_t = xpool.tile([_P, f], f32, tag="x")
                nc.sync.dma_start(x_t[:], x_d.ap()[bi])

                # t7 = min(7*x, 6.9999995)
                t7 = smpool.tile([_P, f], f32, tag="t7")
                nc.vector.tensor_scalar(
                    t7[:], x_t[:], 7.0, _CLAMP_HI, Alu.mult, Alu.min
                )
                # g[p, f, c]: c=0 -> 1.0; c=1..8 -> (t7 >= c)  (fp16: exact 0/1)
                g_t = smpool.tile([_P, f, 9], f16, tag="g")
                nc.gpsimd.memset(g_t[:, :, 0], 1.0)
                nc.vector.tensor_tensor(
                    g_t[:, :, 1:9],
                    t7[:].unsqueeze(2).broadcast_to([_P, f, 8]),
                    iota8[:].unsqueeze(1).broadcast_to([_P, f, 8]),
                    Alu.is_ge,
                )
                j_t = smpool.tile([_P, f], f32, tag="j")
                nc.vector.tensor_reduce(
                    j_t[:], g_t[:, :, 1:9], mybir.AxisListType.X, Alu.add
                )
                # pow[p, f, m] = u^m in fp32;  u = t7 - j
                pw = smpool.tile([_P, f, 4], f32, tag="pow")
                nc.vector.tensor_tensor(pw[:, :, 1], t7[:], j_t[:], Alu.subtract)
                nc.gpsimd.tensor_tensor(pw[:, :, 2], pw[:, :, 1], pw[:, :, 1], Alu.mult)
                nc.gpsimd.tensor_tensor(pw[:, :, 3], pw[:, :, 2], pw[:, :, 1], Alu.mult)
                nc.gpsimd.memset(pw[:, :, 0], 1.0)
                # fp16 high/low split of pow
                pwh = smpool.tile([_P, f, 4], f16, tag="pwh")
                nc.gpsimd.tensor_copy(pwh[:], pw[:])
                pwl = smpool.tile([_P, f, 4], f16, tag="pwl")
                nc.gpsimd.tensor_tensor(pwl[:], pw[:], pwh[:], Alu.subtract)
                # ind_c = g[c] - g[c+1]  (fp16; exact 0/1)
                ind = smpool.tile([_P, f, 8], f16, tag="ind")
                nc.gpsimd.tensor_tensor(
                    ind[:], g_t[:, :, 0:8], g_t[:, :, 1:9], Alu.subtract
                )
                # vh/vl[p, 32c + 4j' + m] = ind * pow_{h,l}
                vh = vpool.tile([_P, f, 8, 4], f16, tag="vh")
                vl = vpool.tile([_P, f, 8, 4], f16, tag="vl")
                ind_b = ind[:].unsqueeze(3).broadcast_to([_P, f, 8, 4])
                nc.vector.tensor_tensor(
                    vh[:], ind_b, pwh[:].unsqueeze(2).broadcast_to([_P, f, 8, 4]),
                    Alu.mult,
                )
                nc.vector.tensor_tensor(
                    vl[:], ind_b, pwl[:].unsqueeze(2).broadcast_to([_P, f, 8, 4]),
                    Alu.mult,
                )
                vh2 = vh[:].rearrange("p f j m -> p (f j m)")
                vl2 = vl[:].rearrange("p f j m -> p (f j m)")

                outb = opool.tile([_P, f * _CH], f32, tag="outb")
                for s4 in range(_NS // 4):
                    # 32x32 block transpose puts K on partitions: row-group
                    # fl holds the 32-wide feature vecs of its 128 elements.
                    # 4 slabs per transpose op to amortize per-op overhead.
                    vht = vtpool.tile([128, 512], f16, tag="vht")
                    vlt = vtpool.tile([128, 512], f16, tag="vlt")
                    nc.vector.transpose(vht[:], vh2[:, 512 * s4 : 512 * (s4 + 1)])
                    nc.vector.transpose(vlt[:], vl2[:, 512 * s4 : 512 * (s4 + 1)])
                    for sl in range(4):
                        s = 4 * s4 + sl
                        c0, c1 = 128 * sl, 128 * sl + 128
                        # 4-bank PSUM quad; each row-group matmuls into its bank
                        quad = psum_o.tile([128, 2048], f32, tag="quad")
                        for fl in range(4):
                            o_ap = quad[:, 512 * fl : 512 * fl + 128]
                            tp = (32 * fl, 0)
                            r0, r1 = 32 * fl, 32 * fl + 32
                            # vh first+second: identical stationary operand
                            # back-to-back lets the weight path cache/elide
                            nc.tensor.matmul(
                                o_ap, vht[r0:r1, c0:c1], ch_t[r0:r1, :],
                                start=True, stop=False, tile_position=tp,
                            )
                            nc.tensor.matmul(
                                o_ap, vht[r0:r1, c0:c1], cl_t[r0:r1, :],
                                start=False, stop=False, tile_position=tp,
                            )
                            nc.tensor.matmul(
                                o_ap, vlt[r0:r1, c0:c1], ch_t[r0:r1, :],
                                start=False, stop=True, tile_position=tp,
                            )
                        # one strided copy: 4 banks -> contiguous 512 cols
                        qv = quad[:].rearrange("p (fl w) -> p fl w", fl=4)[:, :, 0:128]
                        ov = outb[:, 512 * s : 512 * (s + 1)].rearrange(
                            "p (fl c) -> p fl c", fl=4
                        )
                        nc.scalar.copy(ov, qv)
                    # half-block out DMA: start shipping as soon as the first
                    # 8 slabs are copied, overlapping the rest of the block
                    if s4 == _NS // 8 - 1 or s4 == _NS // 4 - 1:
                        h = 0 if s4 == _NS // 8 - 1 else 1
                        hs = _NS // 2
                        nc.sync.dma_start(
                            out_r[bi, :, h * hs : (h + 1) * hs],
                            outb[:, h * hs * 512 : (h + 1) * hs * 512].rearrange(
                                "p (s fl c) -> p s fl c", s=hs, fl=4
                            ),
                        )

    nc.compile()
    return nc


_NC_CACHE = None


def _get_nc():
    global _NC_CACHE
    if _NC_CACHE is None:
        _NC_CACHE = build_nc()
    return _NC_CACHE


# ---------------------------------------------------------------- entrypoint
def kernel(x, W, b):
    from concourse.bass_utils import run_bass_kernel_spmd

    x = np.asarray(x, dtype=np.float32)
    Bsz, T = x.shape
    ch_rep, cl_rep = _build_cmats(W, b)
    nc = _get_nc()

    shards = x.reshape(_NCORES, _NPC)
    in_maps = [
        {"x": _shuffle_x(shards[i]), "cmath": ch_rep, "cmatl": cl_rep}
        for i in range(_NCORES)
    ]
    res = run_bass_kernel_spmd(nc, in_maps, core_ids=list(range(_NCORES)))
    out = np.concatenate([res.results[i]["out"] for i in range(_NCORES)], axis=0)
    return out.reshape(Bsz, T, _CH)


# revision 21
# speedup vs baseline: 1.1646x; 1.1419x over previous
"""Trainium2 Bass kernel for BSplineEncoder.

Reference computation (per scalar x in [0,1)):
    basis = cubic B-spline basis (10 functions, knots uniform on [0,1], clamped)
    out[e, :] = basis(x_e) @ W.T + b          # W: [128, 10], b: [128]

Reformulation: on each of the 7 uniform knot intervals the spline is a cubic,
so with j = floor(7x), u = 7x - j:
    out[e, c] = sum_m u^m * C[j, m, c]
C is precomputed host-side from W (bias folded into the m=0 coefficient of
every interval — valid because exactly one interval indicator fires).
Writing v[e, 4j+m] = ind_j(e) * u^m (32-wide, zero-padded), out = v @ C.

Precision: the matmul runs as a split-fp16 product with fp32 PSUM accumulation
    out = vh@Ch + vl@Ch + vh@Cl       (vh=fp16(v), vl=fp16(v-vh); same for C)
which is fp32-accurate to ~2e-6 while streaming at 1 cycle/row (fp32 matmul
on TRN2 is a 2-instruction replay at 4 cycles/row — 8x slower).

Device pipeline (per core; elements sharded 8 ways on batch):
    x (host-preshuffled) -> [128, F] tiles -> DVE computes vh/vl fp16 feature
    tiles -> DVE 32x32 stream-transpose puts K on partitions -> PE row-tiled
    matmuls (4 groups x 3 accumulating MMs) into a 4-bank PSUM quad ->
    one strided ACT copy per slab -> contiguous 4 MiB DMA per block.

The element->(partition, lane) mapping is chosen so the stream-transpose's
block-local semantics line up with the matmul row groups AND the output DMA
stays fully contiguous; the host reorders x (cheap: 4 MiB) to compensate.
"""

import numpy as np

_P = 128            # SBUF partitions
_F = 64             # x values per partition per block (16 slabs x 4)
_NB = 16            # blocks per core
_NPC = _P * _F * _NB  # 131072 elements per core
_NCORES = 8
_CH = 128           # output channels
_NS = 16            # slabs per block (each slab: 512 elements)
_DEGREE = 3
_N_BASIS = 10
# largest fp32 strictly below 7.0: keeps j <= 6 and u < 1 for any x
_CLAMP_HI = float(np.nextafter(np.float32(7.0), np.float32(0.0)))


# ---------------------------------------------------------------- host math
def _bspline_basis_1d_f64(x, knots, degree=_DEGREE):
    n_knots = knots.shape[0]
    B = ((x[None, :] >= knots[:-1, None]) & (x[None, :] < knots[1:, None])).astype(
        np.float64
    )
    for d in range(1, degree + 1):
        n = n_knots - d - 1
        k_i = knots[:n]
        k_i1 = knots[1 : n + 1]
        k_id = knots[d : d + n]
        k_id1 = knots[d + 1 : d + 1 + n]
        denom1 = k_id - k_i
        denom2 = k_id1 - k_i1
        safe1 = np.where(denom1 != 0, denom1, 1.0)[:, None]
        safe2 = np.where(denom2 != 0, denom2, 1.0)[:, None]
        t1 = np.where(
            denom1[:, None] != 0, (x[None, :] - k_i[:, None]) / safe1 * B[:n], 0.0
        )
        t2 = np.where(
            denom2[:, None] != 0,
            (k_id1[:, None] - x[None, :]) / safe2 * B[1 : n + 1],
            0.0,
        )
        B = t1 + t2
    return B.T  # [N, n_basis]


def _interval_poly_coeffs():
    """M[j][r, m]: basis_{j+r}(x) = sum_m M[j][r,m] * u^m on interval j."""
    n_knots = _N_BASIS + _DEGREE + 1
    base = np.linspace(0.0, 1.0, n_knots - 2 * _DEGREE)
    knots = np.concatenate(
        [np.repeat(base[:1], _DEGREE), base, np.repeat(base[-1:], _DEGREE)]
    )
    M = np.zeros((7, 4, 4))
    us = np.array([0.1, 0.35, 0.65, 0.9])
    V = np.vander(us, 4, increasing=True)
    for j in range(7):
        xs = (j + us) / 7.0
        Bv = _bspline_basis_1d_f64(xs, knots)  # [4, 10]
        coef = np.linalg.solve(V, Bv[:, j : j + 4])  # [m, r]
        M[j] = coef.T  # [r, m]
    return M


def _build_c32(W, b):
    """[32, 128] f32 coeff matrix: rows 4j+m hold sum_r M[j][r,m] W[:, j+r]
    (+ b folded into m=0); rows 28..31 zero."""
    M = _interval_poly_coeffs()
    W64 = np.asarray(W, dtype=np.float64)
    b64 = np.asarray(b, dtype=np.float64)
    C = np.zeros((32, W64.shape[0]))
    for j in range(7):
        for m in range(4):
            C[4 * j + m] = M[j][:, m] @ W64[:, j : j + 4].T
        C[4 * j + 0] += b64
    return C.astype(np.float32)


def _build_cmats(W, b):
    """(ch_rep, cl_rep): fp16 high/low split of C, 4x vertically replicated."""
    C = _build_c32(W, b)
    Ch = C.astype(np.float16)
    Cl = (C - Ch.astype(np.float32)).astype(np.float16)
    return (
        np.ascontiguousarray(np.tile(Ch, (4, 1))),
        np.ascontiguousarray(np.tile(Cl, (4, 1))),
    )


def _shuffle_x(x_core, nb=_NB):
    """Reorder a core's flat x so that compute partition 32*fl+i, free col
    4*s+a holds element e = b*8192 + (32a+i)*64 + s*4 + fl.

    With this mapping the matmul output partition (32a+i) owns a fully
    contiguous 32 KiB range of output rows per block -> ideal out-DMA."""
    # x[b, a, i, s, fl] -> xh[b, (fl,i), (s,a)]
    xs = x_core.reshape(nb, 4, 32, _NS, 4)
    return np.ascontiguousarray(xs.transpose(0, 4, 2, 3, 1).reshape(nb, 128, _F))


# ---------------------------------------------------------------- bass build
def build_nc(nb=_NB):
    """Build the (single-core SPMD) Bass module. Same NEFF runs on all cores."""
    from concourse import bacc, mybir, tile

    Alu = mybir.AluOpType
    f32 = mybir.dt.float32
    f16 = mybir.dt.float16
    npc = _P * _F * nb
    f = _F

    nc = bacc.Bacc("TRN2", target_bir_lowering=False, debug=False)

    x_d = nc.dram_tensor("x", [nb, 128, f], f32, kind="ExternalInput")
    ch_d = nc.dram_tensor("cmath", [128, 128], f16, kind="ExternalInput")
    cl_d = nc.dram_tensor("cmatl", [128, 128], f16, kind="ExternalInput")
    out_d = nc.dram_tensor("out", [npc, _CH], f32, kind="ExternalOutput")
    # interval thresholds 1..8 (g_c = t7 >= c); g_8 is always 0 (t7 < 7)
    iota_d = nc.inline_tensor(
        np.tile(np.arange(1, 9, dtype=np.float32), (128, 1)).copy(), name="iota8"
    )

    # DRAM row for element = b*8192 + p*64 + s*4 + fl; sbuf outb free col is
    # s*512 + fl*128 + ch -> per-block AP [128, 8192], 32 KiB contiguous
    # per partition
    out_r = out_d.ap().rearrange("(b p sfl) c -> b p (sfl c)", b=nb, p=_P)

    with tile.TileContext(nc) as tc:
        with (
            tc.tile_pool(name="consts", bufs=1) as cpool,
            tc.tile_pool(name="xp", bufs=3) as xpool,
            tc.tile_pool(name="sm", bufs=2) as smpool,
            tc.tile_pool(name="vv", bufs=2) as vpool,
            tc.tile_pool(name="vt", bufs=6) as vtpool,
            tc.tile_pool(name="ob", bufs=2) as opool,
            tc.tile_pool(name="pso", bufs=2, space="PSUM") as psum_o,
        ):
            ch_t = cpool.tile([128, 128], f16)
            cl_t = cpool.tile([128, 128], f16)
            iota8 = cpool.tile([128, 8], f32)
            nc.sync.dma_start(ch_t[:], ch_d.ap())
            nc.sync.dma_start(cl_t[:], cl_d.ap())
            nc.sync.dma_start(iota8[:], iota_d.ap())

            for bi in range(nb):
                x_t = xpool.tile([_P, f], f32, tag="x")
                nc.sync.dma_start(x_t[:], x_d.ap()[bi])

                # t7 = min(7*x, 6.9999995)
                t7 = smpool.tile([_P, f], f32, tag="t7")
                nc.vector.tensor_scalar(
                    t7[:], x_t[:], 7.0, _CLAMP_HI, Alu.mult, Alu.min
                )
                # g[p, f, c]: c=0 -> 1.0; c=1..8 -> (t7 >= c)
                g_t = smpool.tile([_P, f, 9], f32, tag="g")
                nc.vector.memset(g_t[:, :, 0], 1.0)
                nc.vector.tensor_tensor(
                    g_t[:, :, 1:9],
                    t7[:].unsqueeze(2).broadcast_to([_P, f, 8]),
                    iota8[:].unsqueeze(1).broadcast_to([_P, f, 8]),
                    Alu.is_ge,
                )
                j_t = smpool.tile([_P, f], f32, tag="j")
                nc.vector.tensor_reduce(
                    j_t[:], g_t[:, :, 1:9], mybir.AxisListType.X, Alu.add
                )
                # pow[p, f, m] = u^m in fp32;  u = t7 - j
                pw = smpool.tile([_P, f, 4], f32, tag="pow")
                nc.vector.tensor_tensor(pw[:, :, 1], t7[:], j_t[:], Alu.subtract)
                nc.gpsimd.tensor_tensor(pw[:, :, 2], pw[:, :, 1], pw[:, :, 1], Alu.mult)
                nc.gpsimd.tensor_tensor(pw[:, :, 3], pw[:, :, 2], pw[:, :, 1], Alu.mult)
                nc.gpsimd.memset(pw[:, :, 0], 1.0)
                # fp16 high/low split of pow
                pwh = smpool.tile([_P, f, 4], f16, tag="pwh")
                nc.gpsimd.tensor_copy(pwh[:], pw[:])
                pwl = smpool.tile([_P, f, 4], f16, tag="pwl")
                nc.gpsimd.tensor_tensor(pwl[:], pw[:], pwh[:], Alu.subtract)
                # ind_c = g[c] - g[c+1]  (fp16; exact 0/1)
                ind = smpool.tile([_P, f, 8], f16, tag="ind")
                nc.gpsimd.tensor_tensor(
                    ind[:], g_t[:, :, 0:8], g_t[:, :, 1:9], Alu.subtract
                )
                # vh/vl[p, 32c + 4j' + m] = ind * pow_{h,l}
                vh = vpool.tile([_P, f, 8, 4], f16, tag="vh")
                vl = vpool.tile([_P, f, 8, 4], f16, tag="vl")
                ind_b = ind[:].unsqueeze(3).broadcast_to([_P, f, 8, 4])
                nc.vector.tensor_tensor(
                    vh[:], ind_b, pwh[:].unsqueeze(2).broadcast_to([_P, f, 8, 4]),
                    Alu.mult,
                )
                nc.vector.tensor_tensor(
                    vl[:], ind_b, pwl[:].unsqueeze(2).broadcast_to([_P, f, 8, 4]),
                    Alu.mult,
                )
                vh2 = vh[:].rearrange("p f j m -> p (f j m)")
                vl2 = vl[:].rearrange("p f j m -> p (f j m)")

                outb = opool.tile([_P, f * _CH], f32, tag="outb")
                for s4 in range(_NS // 4):
                    # 32x32 block transpose puts K on partitions: row-group
                    # fl holds the 32-wide feature vecs of its 128 elements.
                    # 4 slabs per transpose op to amortize per-op overhead.
                    vht = vtpool.tile([128, 512], f16, tag="vht")
                    vlt = vtpool.tile([128, 512], f16, tag="vlt")
                    nc.vector.transpose(vht[:], vh2[:, 512 * s4 : 512 * (s4 + 1)])
                    nc.vector.transpose(vlt[:], vl2[:, 512 * s4 : 512 * (s4 + 1)])
                    for sl in range(4):
                        s = 4 * s4 + sl
                        c0, c1 = 128 * sl, 128 * sl + 128
                        # 4-bank PSUM quad; each row-group matmuls into its bank
                        quad = psum_o.tile([128, 2048], f32, tag="quad")
                        for fl in range(4):
                            o_ap = quad[:, 512 * fl : 512 * fl + 128]
                            tp = (32 * fl, 0)
                            r0, r1 = 32 * fl, 32 * fl + 32
                            # vh first+second: identical stationary operand
                            # back-to-back lets the weight path cache/elide
                            nc.tensor.matmul(
                                o_ap, vht[r0:r1, c0:c1], ch_t[r0:r1, :],
                                start=True, stop=False, tile_position=tp,
                            )
                            nc.tensor.matmul(
                                o_ap, vht[r0:r1, c0:c1], cl_t[r0:r1, :],
                                start=False, stop=False, tile_position=tp,
                            )
                            nc.tensor.matmul(
                                o_ap, vlt[r0:r1, c0:c1], ch_t[r0:r1, :],
                                start=False, stop=True, tile_position=tp,
                            )
                        # one strided copy: 4 banks -> contiguous 512 cols
                        qv = quad[:].rearrange("p (fl w) -> p fl w", fl=4)[:, :, 0:128]
                        ov = outb[:, 512 * s : 512 * (s + 1)].rearrange(
                            "p (fl c) -> p fl c", fl=4
                        )
                        nc.scalar.copy(ov, qv)
                nc.sync.dma_start(out_r[bi], outb[:])

    nc.compile()
    return nc


_NC_CACHE = None


def _get_nc():
    global _NC_CACHE
    if _NC_CACHE is None:
        _NC_CACHE = build_nc()
    return _NC_CACHE


# ---------------------------------------------------------------- entrypoint
def kernel(x, W, b):
    from concourse.bass_utils import run_bass_kernel_spmd

    x = np.asarray(x, dtype=np.float32)
    Bsz, T = x.shape
    ch_rep, cl_rep = _build_cmats(W, b)
    nc = _get_nc()

    shards = x.reshape(_NCORES, _NPC)
    in_maps = [
        {"x": _shuffle_x(shards[i]), "cmath": ch_rep, "cmatl": cl_rep}
        for i in range(_NCORES)
    ]
    res = run_bass_kernel_spmd(nc, in_maps, core_ids=list(range(_NCORES)))
    out = np.concatenate([res.results[i]["out"] for i in range(_NCORES)], axis=0)
    return out.reshape(Bsz, T, _CH)
